# revision 7
# baseline (speedup 1.0000x reference)
"""Trainium2 Bass kernel for nn_MixMobileBlock (B=32, C=512, H=W=32, NH=8, ER=4).

Data-parallel over 8 NeuronCores: 4 examples per core. Everything is kept
channel-major on chip; LN gamma/beta are folded into the following matmul
weights host-side; depthwise-conv + residual + 2x2 avgpool are fused into a
single stride-2 4x4 depthwise conv computed with 16 DVE MAC taps.
"""

import numpy as np
import ml_dtypes

import concourse.bass as bass
import concourse.mybir as mybir
import concourse.tile as tile
from concourse import bacc

F32 = mybir.dt.float32
BF16 = mybir.dt.bfloat16
Alu = mybir.AluOpType
Act = mybir.ActivationFunctionType
AX = mybir.AxisListType

B, C, H, W = 32, 512, 32, 32
NH = 8
DH = C // NH          # 64
ER = 4
N1 = 256              # tokens after pool (16x16)
N2 = 1024             # positions after convT (32x32) = 4 planes x 256
EPS = 1e-6
NCORES = 8
CT = C // 128         # 4 channel tiles
NEG = -1e12


def _ap_shift(t_ap, off, dims):
    """AP at element offset `off` into tile view t_ap with free dims `dims`."""
    return bass.AP(tensor=t_ap.tensor, offset=t_ap.offset + off,
                   ap=[t_ap.ap[0]] + [list(d) for d in dims])


def build_nc(ex, gelu_mode="hw", stop_after=None, variant="base"):
    nc = bacc.Bacc(None, target_bir_lowering=False)

    x_d = nc.dram_tensor("x", [ex, C, H, W], F32, kind="ExternalInput")
    mask_d = nc.dram_tensor("mask", [ex, C, DH], F32, kind="ExternalInput")
    k4w_d = nc.dram_tensor("k4w", [CT, 128, 16], F32, kind="ExternalInput")
    dwb_d = nc.dram_tensor("dwb", [CT, 128], F32, kind="ExternalInput")
    qkvw_d = nc.dram_tensor("qkvw", [CT, 128, 3 * C], BF16, kind="ExternalInput")
    qkvb_d = nc.dram_tensor("qkvb", [12, 128], F32, kind="ExternalInput")
    tempv_d = nc.dram_tensor("tempv", [CT, 128], F32, kind="ExternalInput")
    projw_d = nc.dram_tensor("projw", [CT, 128, C], BF16, kind="ExternalInput")
    projb_d = nc.dram_tensor("projb", [CT, 128], F32, kind="ExternalInput")
    ctw_d = nc.dram_tensor("ctw", [4, CT, 128, C], BF16, kind="ExternalInput")
    ctb_d = nc.dram_tensor("ctb", [CT, 128], F32, kind="ExternalInput")
    pw1w_d = nc.dram_tensor("pw1w", [CT, 128, ER * C], BF16, kind="ExternalInput")
    pw1b_d = nc.dram_tensor("pw1b", [16, 128], F32, kind="ExternalInput")
    pw2w_d = nc.dram_tensor("pw2w", [16, 128, C], BF16, kind="ExternalInput")
    pw2b_d = nc.dram_tensor("pw2b", [CT, 128], F32, kind="ExternalInput")
    out_d = nc.dram_tensor("out", [ex, C, H, W], F32, kind="ExternalOutput")

    with tile.TileContext(nc) as tc:
        with tc.tile_pool(name="wts", bufs=1) as wts, \
             tc.tile_pool(name="work", bufs=2) as work, \
             tc.tile_pool(name="big", bufs=1) as big, \
             tc.tile_pool(name="ps", bufs=6, space="PSUM") as ps, \
             tc.tile_pool(name="psr", bufs=2, space="PSUM") as psr:

            # ---- load weights (once) ----
            def wload(dram, shape3, dt, tag):
                t = wts.tile([128, shape3[0], shape3[2]], dt, tag=tag)
                nc.sync.dma_start(t[:], dram[:].rearrange("k p n -> p k n"))
                return t

            qkvw = wload(qkvw_d, [CT, 128, 3 * C], BF16, "qkvw")
            projw = wload(projw_d, [CT, 128, C], BF16, "projw")
            pw1w = wload(pw1w_d, [CT, 128, ER * C], BF16, "pw1w")
            pw2w = wload(pw2w_d, [16, 128, C], BF16, "pw2w")
            ctw = wts.tile([128, 4, CT, C], BF16)
            nc.sync.dma_start(ctw[:], ctw_d[:].rearrange("q k p n -> p q k n"))
            k4w = wload(k4w_d, [CT, 128, 16], F32, "k4w")

            def cload(dram, n, tag):
                t = wts.tile([128, n], F32, tag=tag)
                nc.sync.dma_start(t[:], dram[:].rearrange("k p -> p k"))
                return t

            dwb = cload(dwb_d, CT, "dwb")
            qkvb = cload(qkvb_d, 12, "qkvb")
            tempv = cload(tempv_d, CT, "tempv")
            projb = cload(projb_d, CT, "projb")
            ctb = cload(ctb_d, CT, "ctb")
            pw1b = cload(pw1b_d, 16, "pw1b")
            pw2b = cload(pw2b_d, CT, "pw2b")

            ident = wts.tile([128, 128], BF16)
            from concourse.masks import make_identity
            make_identity(nc, ident[:])
            onesS = wts.tile([128, 1], BF16)
            nc.vector.memset(onesS[:], 1.0 / C)
            eps1 = wts.tile([1, 1], F32)
            nc.vector.memset(eps1[:], EPS)

            def bcast(dst, row):
                if variant in ("nopb", "allsafe"):
                    rv = row[:]
                    src = bass.AP(tensor=rv.tensor, offset=rv.offset,
                                  ap=[[0, 128]] + [list(d) for d in rv.ap[1:]])
                    nc.sync.dma_start(dst, src)
                else:
                    nc.gpsimd.partition_broadcast(dst, row, channels=128)

            def gcopy(dst, src):
                if variant in ("nogps", "allsafe"):
                    nc.vector.tensor_copy(dst, src)
                else:
                    nc.gpsimd.tensor_copy(dst, src)

            def gmul(dst, a, b):
                if variant in ("nogps", "allsafe"):
                    nc.vector.tensor_tensor(out=dst, in0=a, in1=b, op=Alu.mult)
                else:
                    nc.gpsimd.tensor_tensor(out=dst, in0=a, in1=b, op=Alu.mult)

            # two persistent padded-image tiles; borders zeroed once, reused
            pads = []
            for i in range(2):
                pt = wts.tile([128, 34, 34], F32, tag=f"pad{i}")
                nc.gpsimd.memset(pt[:], 0.0)
                pads.append(pt)

            for b in range(ex):
                # ============ stage A: fused dwconv+residual+avgpool ========
                ybf = work.tile([128, CT, N1], BF16, tag="ybf")
                for t in range(CT):
                    pad = pads[(b * CT + t) % 2]
                    nc.sync.dma_start(pad[:, 1:33, 1:33], x_d[b, t * 128:(t + 1) * 128])
                    y = work.tile([128, N1], F32, tag="y")
                    for m in range(4):
                        for n in range(4):
                            idx = m * 4 + n
                            src = _ap_shift(pad[:], m * 34 + n, [[68, 16], [2, 16]])
                            if idx == 0:
                                nc.vector.tensor_scalar(
                                    out=y[:], in0=src,
                                    scalar1=k4w[:, t, 0:1], scalar2=dwb[:, t:t + 1],
                                    op0=Alu.mult, op1=Alu.add)
                            else:
                                nc.vector.scalar_tensor_tensor(
                                    out=y[:], in0=src, scalar=k4w[:, t, idx:idx + 1],
                                    in1=y[:], op0=Alu.mult, op1=Alu.add)
                    gcopy(ybf[:, t, :], y[:])

                if stop_after == "A":
                    continue
                # ============ LN1 stats (over channels) =====================
                mu_ps = psr.tile([1, N1], F32, tag="row")
                m2_ps = psr.tile([1, N1], F32, tag="row")
                for t in range(CT):
                    sq = work.tile([128, N1], BF16, tag="sq")
                    gmul(sq[:], ybf[:, t, :], ybf[:, t, :])
                    nc.tensor.matmul(mu_ps[:], onesS[:], ybf[:, t, :],
                                     start=(t == 0), stop=(t == CT - 1))
                    nc.tensor.matmul(m2_ps[:], onesS[:], sq[:],
                                     start=(t == 0), stop=(t == CT - 1))

                murow = work.tile([1, N1], F32, tag="murow")
                nc.vector.tensor_copy(murow[:], mu_ps[:])
                var = work.tile([1, N1], F32, tag="var")
                # var = m2 - mu^2  (clamped at 0)
                nc.vector.tensor_tensor(out=var[:], in0=murow[:], in1=murow[:],
                                        op=Alu.mult)
                nc.vector.tensor_tensor(out=var[:], in0=m2_ps[:], in1=var[:],
                                        op=Alu.subtract)
                nc.vector.tensor_scalar_max(out=var[:], in0=var[:], scalar1=0.0)
                nc.scalar.activation(out=var[:], in_=var[:], func=Act.Sqrt,
                                     bias=eps1[:], scale=1.0)
                isrow = work.tile([1, N1], F32, tag="isrow")
                nc.vector.reciprocal(isrow[:], var[:])

                mu1bc = work.tile([128, N1], F32, tag="mu1bc", bufs=1)
                bcast(mu1bc[:], murow[:])
                is1bc = work.tile([128, N1], F32, tag="is1bc", bufs=1)
                bcast(is1bc[:], isrow[:])

                z1 = big.tile([128, CT, N1], BF16, tag="z1")
                for t in range(CT):
                    zt = work.tile([128, N1], BF16, tag="zt")
                    nc.vector.tensor_tensor(out=zt[:], in0=ybf[:, t, :],
                                            in1=mu1bc[:], op=Alu.subtract)
                    nc.vector.tensor_tensor(out=z1[:, t, :], in0=zt[:],
                                            in1=is1bc[:], op=Alu.mult)

                if stop_after == "ln1":
                    continue
                # ============ QKV matmul ====================================
                qk = []   # fp32 q,k tiles (m 0..7)
                vt = work.tile([128, CT, N1], BF16, tag="vt")
                for m in range(12):
                    mm_ps = ps.tile([128, 512], F32, tag="mm")
                    for k in range(CT):
                        nc.tensor.matmul(mm_ps[:, 0:N1],
                                         qkvw[:, k, m * 128:(m + 1) * 128],
                                         z1[:, k, :],
                                         start=(k == 0), stop=(k == CT - 1))
                    if m < 8:
                        qf = work.tile([128, N1], F32, tag=f"qk{m % 4}_{m // 4}", bufs=1)
                        nc.vector.tensor_scalar_add(out=qf[:], in0=mm_ps[:, 0:N1],
                                                    scalar1=qkvb[:, m:m + 1])
                        qk.append(qf)
                    else:
                        nc.vector.tensor_scalar_add(out=vt[:, m - 8, :],
                                                    in0=mm_ps[:, 0:N1],
                                                    scalar1=qkvb[:, m:m + 1])

                if stop_after == "qkv":
                    continue
                # ============ attention (per head-pair tile) ================
                def normalize(src, tag):
                    ssq = work.tile([128, 1], F32, tag=f"ssq{tag}")
                    scr = work.tile([128, N1], F32, tag="scr")
                    nc.scalar.activation(out=scr[:], in_=src[:], func=Act.Square,
                                         accum_out=ssq[:])
                    nc.scalar.activation(out=ssq[:], in_=ssq[:], func=Act.Sqrt)
                    nc.vector.tensor_scalar_max(out=ssq[:], in0=ssq[:], scalar1=1e-12)
                    rinv = work.tile([128, 1], F32, tag=f"rinv{tag}")
                    nc.vector.reciprocal(rinv[:], ssq[:])
                    hat = work.tile([128, N1], BF16, tag=f"hat{tag}")
                    nc.vector.tensor_scalar_mul(out=hat[:], in0=src[:], scalar1=rinv[:])
                    return hat

                O = work.tile([128, CT, N1], BF16, tag="O")
                for t in range(CT):
                    qhat = normalize(qk[t], "q")
                    khat = normalize(qk[4 + t], "k")
                    qT = work.tile([128, 2, 128], BF16, tag="qT")
                    kT = work.tile([128, 2, 128], BF16, tag="kT")
                    for h in range(2):
                        for (src, dst) in ((qhat, qT), (khat, kT)):
                            tr_ps = ps.tile([128, 128], BF16, tag="mm")
                            nc.tensor.transpose(tr_ps[:], src[:, h * 128:(h + 1) * 128],
                                                ident[:])
                            nc.vector.tensor_copy(dst[:, h, :], tr_ps[:])

                    s_ps = ps.tile([128, 128], F32, tag="mm")
                    for h in range(2):
                        nc.tensor.matmul(s_ps[:], qT[:, h, :], kT[:, h, :],
                                         start=(h == 0), stop=(h == 1))

                    msb = work.tile([128, DH], F32, tag="msb")
                    nc.sync.dma_start(msb[:], mask_d[b, t * 128:(t + 1) * 128])
                    mad = work.tile([128, 128], F32, tag="mad")
                    (nc.vector.memset(mad[:], NEG) if variant in ("nogps", "allsafe")
                     else nc.gpsimd.memset(mad[:], NEG))
                    for h in range(2):
                        nc.vector.tensor_scalar(
                            out=mad[h * 64:(h + 1) * 64, h * 64:(h + 1) * 64],
                            in0=msb[h * 64:(h + 1) * 64, :],
                            scalar1=0.2, scalar2=NEG, op0=Alu.is_lt, op1=Alu.mult)

                    apre = work.tile([128, 128], F32, tag="apre")
                    nc.vector.scalar_tensor_tensor(out=apre[:], in0=s_ps[:],
                                                   scalar=tempv[:, t:t + 1], in1=mad[:],
                                                   op0=Alu.mult, op1=Alu.add)
                    nmax = work.tile([128, 1], F32, tag="nmax")
                    nc.vector.tensor_reduce(out=nmax[:], in_=apre[:], axis=AX.X,
                                            op=Alu.max, negate=True)
                    expv = work.tile([128, 128], F32, tag="expv")
                    rsum = work.tile([128, 1], F32, tag="rsum")
                    nc.scalar.activation(out=expv[:], in_=apre[:], func=Act.Exp,
                                         bias=nmax[:], accum_out=rsum[:])
                    rs_i = work.tile([128, 1], F32, tag="rs_i")
                    nc.vector.reciprocal(rs_i[:], rsum[:])
                    attn = work.tile([128, 128], BF16, tag="attn")
                    nc.vector.tensor_scalar_mul(out=attn[:], in0=expv[:], scalar1=rs_i[:])

                    at_ps = ps.tile([128, 128], BF16, tag="mm")
                    nc.tensor.transpose(at_ps[:], attn[:], ident[:])
                    attnT = work.tile([128, 128], BF16, tag="attnT")
                    nc.vector.tensor_copy(attnT[:], at_ps[:])

                    o_ps = ps.tile([128, 512], F32, tag="mm")
                    nc.tensor.matmul(o_ps[:, 0:N1], attnT[:], vt[:, t, :],
                                     start=True, stop=True)
                    nc.vector.tensor_copy(O[:, t, :], o_ps[:, 0:N1])

                if stop_after == "attn":
                    continue
                # ============ proj (x2 folded) ==============================
                P = work.tile([128, CT, N1], BF16, tag="P")
                for m in range(CT):
                    mm_ps = ps.tile([128, 512], F32, tag="mm")
                    for k in range(CT):
                        nc.tensor.matmul(mm_ps[:, 0:N1],
                                         projw[:, k, m * 128:(m + 1) * 128],
                                         O[:, k, :], start=(k == 0), stop=(k == CT - 1))
                    nc.vector.tensor_scalar_add(out=P[:, m, :], in0=mm_ps[:, 0:N1],
                                                scalar1=projb[:, m:m + 1])

                if stop_after == "proj":
                    continue
                # ============ convT: 4 plane matmuls ========================
                ubf = big.tile([128, CT, 4, N1], BF16, tag="ubf")
                for pq in range(4):
                    for m in range(CT):
                        mm_ps = ps.tile([128, 512], F32, tag="mm")
                        for k in range(CT):
                            nc.tensor.matmul(mm_ps[:, 0:N1],
                                             ctw[:, pq, k, m * 128:(m + 1) * 128],
                                             P[:, k, :],
                                             start=(k == 0), stop=(k == CT - 1))
                        nc.vector.tensor_scalar_add(out=ubf[:, m, pq, :],
                                                    in0=mm_ps[:, 0:N1],
                                                    scalar1=ctb[:, m:m + 1])

                if stop_after == "convt":
                    continue
                # ============ LN2 stats + z2 (per plane) ====================
                z2 = big.tile([128, CT, 4, N1], BF16, tag="z2")
                for pq in range(4):
                    mu_ps2 = psr.tile([1, N1], F32, tag="row")
                    m2_ps2 = psr.tile([1, N1], F32, tag="row")
                    for k in range(CT):
                        usq = work.tile([128, N1], BF16, tag="usq")
                        gmul(usq[:], ubf[:, k, pq, :], ubf[:, k, pq, :])
                        nc.tensor.matmul(mu_ps2[:], onesS[:], ubf[:, k, pq, :],
                                         start=(k == 0), stop=(k == CT - 1))
                        nc.tensor.matmul(m2_ps2[:], onesS[:], usq[:],
                                         start=(k == 0), stop=(k == CT - 1))
                    murow2 = work.tile([1, N1], F32, tag="murow2")
                    nc.vector.tensor_copy(murow2[:], mu_ps2[:])
                    var2 = work.tile([1, N1], F32, tag="var2")
                    nc.vector.tensor_tensor(out=var2[:], in0=murow2[:], in1=murow2[:],
                                            op=Alu.mult)
                    nc.vector.tensor_tensor(out=var2[:], in0=m2_ps2[:], in1=var2[:],
                                            op=Alu.subtract)
                    nc.vector.tensor_scalar_max(out=var2[:], in0=var2[:], scalar1=0.0)
                    nc.scalar.activation(out=var2[:], in_=var2[:], func=Act.Sqrt,
                                         bias=eps1[:], scale=1.0)
                    isrow2 = work.tile([1, N1], F32, tag="isrow2")
                    nc.vector.reciprocal(isrow2[:], var2[:])
                    mu2bc = work.tile([128, N1], F32, tag="mu2bc")
                    bcast(mu2bc[:], murow2[:])
                    is2bc = work.tile([128, N1], F32, tag="is2bc")
                    bcast(is2bc[:], isrow2[:])
                    for k in range(CT):
                        zt2 = work.tile([128, N1], BF16, tag="zt2")
                        nc.vector.tensor_tensor(out=zt2[:], in0=ubf[:, k, pq, :],
                                                in1=mu2bc[:], op=Alu.subtract)
                        nc.vector.tensor_tensor(out=z2[:, k, pq, :], in0=zt2[:],
                                                in1=is2bc[:], op=Alu.mult)

                if stop_after == "ln2":
                    continue
                # ============ MLP: pw1 -> gelu -> pw2 -> +inp ===============
                Ht = big.tile([128, 16, N2], BF16, tag="H")
                z2v = z2[:].rearrange("p k q n -> p k (q n)")
                for mo in range(16):
                    for half in range(2):
                        h_ps = ps.tile([128, 512], F32, tag="mm")
                        for sub in range(2):
                            pq = half * 2 + sub
                            for k in range(CT):
                                nc.tensor.matmul(
                                    h_ps[:, sub * N1:(sub + 1) * N1],
                                    pw1w[:, k, mo * 128:(mo + 1) * 128],
                                    z2[:, k, pq, :],
                                    start=(k == 0), stop=(k == CT - 1))
                        hslc = Ht[:, mo, half * 512:(half + 1) * 512]
                        if gelu_mode == "hw":
                            nc.scalar.activation(out=hslc, in_=h_ps[:], func=Act.Gelu,
                                                 bias=pw1b[:, mo:mo + 1], scale=1.0)
                        else:
                            # tanh-approx gelu for CoreSim verification
                            pre = work.tile([128, 512], F32, tag="pre")
                            nc.vector.tensor_scalar_add(out=pre[:], in0=h_ps[:],
                                                        scalar1=pw1b[:, mo:mo + 1])
                            t2_ = work.tile([128, 512], F32, tag="t2_")
                            nc.vector.tensor_tensor(out=t2_[:], in0=pre[:], in1=pre[:],
                                                    op=Alu.mult)
                            nc.vector.tensor_scalar(out=t2_[:], in0=t2_[:],
                                                    scalar1=0.044715, scalar2=1.0,
                                                    op0=Alu.mult, op1=Alu.add)
                            nc.vector.tensor_tensor(out=t2_[:], in0=t2_[:], in1=pre[:],
                                                    op=Alu.mult)
                            nc.scalar.activation(out=t2_[:], in_=t2_[:], func=Act.Tanh,
                                                 scale=0.7978845608028654)
                            nc.vector.tensor_scalar(out=t2_[:], in0=t2_[:],
                                                    scalar1=1.0, scalar2=0.5,
                                                    op0=Alu.add, op1=Alu.mult)
                            nc.vector.tensor_tensor(out=hslc, in0=t2_[:], in1=pre[:],
                                                    op=Alu.mult)

                if stop_after == "pw1":
                    continue
                for mo in range(CT):
                    inp = work.tile([128, 4, N1], F32, tag="inp")
                    xv = x_d[b, mo * 128:(mo + 1) * 128].rearrange(
                        "c (i p) (j q) -> c p q i j", p=2, q=2)
                    for pq in range(4):
                        nc.sync.dma_start(
                            inp[:, pq, :].rearrange("c (i j) -> c i j", i=16),
                            xv[:, pq // 2, pq % 2])
                    for half in range(2):
                        g_ps = ps.tile([128, 512], F32, tag="mm")
                        for k in range(16):
                            nc.tensor.matmul(
                                g_ps[:],
                                pw2w[:, k, mo * 128:(mo + 1) * 128],
                                Ht[:, k, half * 512:(half + 1) * 512],
                                start=(k == 0), stop=(k == 15))
                        fin = work.tile([128, 512], F32, tag="fin")
                        nc.vector.scalar_tensor_tensor(
                            out=fin[:], in0=g_ps[:], scalar=pw2b[:, mo:mo + 1],
                            in1=inp[:].rearrange("c q n -> c (q n)")[
                                :, half * 512:(half + 1) * 512],
                            op0=Alu.add, op1=Alu.add)
                        ov = out_d[b, mo * 128:(mo + 1) * 128].rearrange(
                            "c (i p) (j q) -> c p q i j", p=2, q=2)
                        for sub in range(2):
                            pq = half * 2 + sub
                            nc.sync.dma_start(
                                ov[:, pq // 2, pq % 2],
                                fin[:, sub * N1:(sub + 1) * N1].rearrange(
                                    "c (i j) -> c i j", i=16))

    nc.compile()
    return nc


def prep_weights(inputs):
    """Host-side weight preprocessing (folding + layout). Returns dict of np arrays."""
    f32 = np.float32
    bf16 = ml_dtypes.bfloat16
    dw_w = np.asarray(inputs["dw_w"], f32)          # [C,1,3,3]
    dw_b = np.asarray(inputs["dw_b"], f32)
    ln1_g = np.asarray(inputs["ln1_g"], f32)
    ln1_b = np.asarray(inputs["ln1_b"], f32)
    qkv_w = np.asarray(inputs["qkv_w"], f32)        # [3C, C]
    qkv_b = np.asarray(inputs["qkv_b"], f32)
    temp = np.asarray(inputs["temp"], f32).reshape(NH)
    proj_w = np.asarray(inputs["proj_w"], f32)
    proj_b = np.asarray(inputs["proj_b"], f32)
    ct_w = np.asarray(inputs["ct_w"], f32)          # [C,C,2,2] (in,out,p,q)
    ct_b = np.asarray(inputs["ct_b"], f32)
    ln2_g = np.asarray(inputs["ln2_g"], f32)
    ln2_b = np.asarray(inputs["ln2_b"], f32)
    pw1_w = np.asarray(inputs["pw1_w"], f32)        # [4C, C]
    pw1_b = np.asarray(inputs["pw1_b"], f32)
    pw2_w = np.asarray(inputs["pw2_w"], f32)        # [C, 4C]
    pw2_b = np.asarray(inputs["pw2_b"], f32)

    # fused stride-2 4x4 kernel: K[m,n] = sum_{p,q in 0,1} w3[m-p, n-q]; /4; +1/4 id
    w3 = dw_w[:, 0]                                  # [C,3,3]
    K4 = np.zeros((C, 4, 4), f32)
    for p in range(2):
        for q in range(2):
            K4[:, p:p + 3, q:q + 3] += w3
    K4 /= 4.0
    K4[:, 1:3, 1:3] += 0.25
    k4w = K4.reshape(C, 16).reshape(CT, 128, 16)

    def tile_lhsT(wT, nk):  # wT [C_in, M] -> [nk, 128, M]
        return np.ascontiguousarray(wT.reshape(nk, 128, -1)).astype(bf16)

    qkv_eff = qkv_w * ln1_g[None, :]
    qkvw = tile_lhsT(qkv_eff.T, CT)                  # [4,128,1536]
    qkvb = (qkv_w @ ln1_b + qkv_b).reshape(12, 128).astype(f32)
    tempv = np.repeat(temp, DH).reshape(CT, 128).astype(f32)
    projw = tile_lhsT((2.0 * proj_w).T, CT)
    projb = (2.0 * proj_b).reshape(CT, 128).astype(f32)
    ctw = np.stack([tile_lhsT(ct_w[:, :, pq // 2, pq % 2], CT) for pq in range(4)])
    ctb = ct_b.reshape(CT, 128).astype(f32)
    pw1_eff = pw1_w * ln2_g[None, :]
    pw1w = tile_lhsT(pw1_eff.T, CT)                  # [4,128,2048]
    pw1b = (pw1_w @ ln2_b + pw1_b).reshape(16, 128).astype(f32)
    pw2w = tile_lhsT(pw2_w.T, 16)                    # [16,128,512]
    pw2b = pw2_b.reshape(CT, 128).astype(f32)

    return dict(k4w=k4w, dwb=dw_b.reshape(CT, 128).astype(f32), qkvw=qkvw,
                qkvb=qkvb, tempv=tempv, projw=projw, projb=projb, ctw=ctw,
                ctb=ctb, pw1w=pw1w, pw1b=pw1b, pw2w=pw2w, pw2b=pw2b)


_NC_CACHE = {}


def kernel(**inputs) -> np.ndarray:
    from concourse.bass_utils import run_bass_kernel_spmd

    x = np.asarray(inputs["x"], np.float32)          # [B,C,H,W]
    mask_u = np.asarray(inputs["mask_u"], np.float32)  # [B,NH,DH,DH]
    wd = prep_weights(inputs)

    ex = B // NCORES
    key = (ex, "hw")
    if key not in _NC_CACHE:
        _NC_CACHE[key] = build_nc(ex, gelu_mode="hw")
    nc = _NC_CACHE[key]

    in_maps = []
    for c in range(NCORES):
        sl = slice(c * ex, (c + 1) * ex)
        m = dict(wd)
        m["x"] = np.ascontiguousarray(x[sl])
        m["mask"] = np.ascontiguousarray(
            mask_u[sl].reshape(ex, C, DH))
        in_maps.append(m)

    res = run_bass_kernel_spmd(nc, in_maps, core_ids=list(range(NCORES)))
    out = np.concatenate([r["out"] for r in res.results], axis=0)
    return out.astype(np.float32)


if __name__ == "__main__":
    pass
